# revision 21
# baseline (speedup 1.0000x reference)
"""Trainium2 Bass kernel for BitLTIInjection (BitNet-style fake-quantized linear
+ LTI injection):

    A_eff = 0.99*tanh(A_raw)
    e_q   = per-token absmax int8 fake quant of e
    W_q   = absmean ternary fake quant of W
    out   = A_eff*h + e_q @ W_q.T + block_out

Strategy v2: data-parallel over B*T across 8 cores; W replicated.

The quantized matmul runs in fp8e4 with MatmulPerfMode.DoubleRow (K=256 per
matmul, 2x MAC rate).  W_q in {-1,0,1} is exact in e4m3; e_q (ints in
[-128,127]) is RNE-cast to e4m3 which introduces a small, offline-verified
error (rel err ~1.5e-2 < 2e-2 gate).  Rounding uses the f32 magic-number
trick (x + 1.5*2^23 - 1.5*2^23 = RNE-to-integer).

Schedule: all transposes run on the PE (fp8 transpose-mode into PSUM, with
the clip/cast fused into the PSUM->SBUF DVE evacuation), keeping the DMA
fabric for pure HBM streaming.  The main loop is ob-major over (token-block,
out-column-block) units of [128x512] so matmuls start as soon as the first 4
W column-tiles are ternarized (~65us) instead of after full W prep.  9 of 16
W f32 tiles stay resident in SBUF after the absmean pass; 7 are re-loaded.
"""

import numpy as np

import concourse.bass as bass
import concourse.mybir as mybir
import concourse.tile as tile
from concourse.tile_rust import add_dep_helper
from concourse.bass import ts
from concourse.bass_utils import run_bass_kernel_spmd

P = 128
MAGIC = 12582912.0  # 1.5 * 2**23: forces RNE-to-integer in f32
EPS = 1e-5
N_CORES = 8
F32 = mybir.dt.float32
BF16 = mybir.dt.bfloat16
FP8 = mybir.dt.float8e4
MM_N = 512   # psum tile free dim (one f32 bank)
N_WST = 9    # W f32 tiles kept resident after absmean pass
BO_PRE = 5   # block_out quarter-tile prefetch depth (units)
DR = mybir.MatmulPerfMode.DoubleRow


def build_kernel_body(tc: tile.TileContext, io: dict, Tc: int, D: int, with_h: bool):
    nc = tc.nc
    n_tb = Tc // P     # token blocks per core
    n_dc = D // P      # contraction chunks of 128
    n_ob = D // MM_N   # output column blocks
    n_wt = D // P      # weight row tiles (j)
    n_pr = n_dc // 2   # DoubleRow K-pairs

    e_d = io["e"]
    bo_d = io["bo"]
    w_d = io["w"]
    eye_d = io["eye"]
    out_d = io["out"]

    with (
        tc.tile_pool(name="wst", bufs=N_WST) as wst_pool,
        tc.tile_pool(name="wre", bufs=2) as wre_pool,
        tc.tile_pool(name="wq8", bufs=1) as wq8_pool,
        tc.tile_pool(name="wtmp", bufs=1) as wtmp_pool,
        tc.tile_pool(name="wq8pre", bufs=2) as wq8pre_pool,
        tc.tile_pool(name="ef", bufs=2) as ef_pool,
        tc.tile_pool(name="q8e", bufs=2) as q8e_pool,
        tc.tile_pool(name="e8T", bufs=n_tb) as e8T_pool,
        tc.tile_pool(name="bo", bufs=6) as bo_pool,
        tc.tile_pool(name="scal", bufs=1) as scal_pool,
        tc.tile_pool(name="st", bufs=3) as st_pool,
        tc.tile_pool(name="deq", bufs=n_tb) as deq_pool,
        tc.tile_pool(name="mm_ps", bufs=2, space="PSUM") as mm_ps_pool,
        tc.tile_pool(name="et_ps", bufs=2, space="PSUM") as et_ps_pool,
        tc.tile_pool(name="wt_ps", bufs=1, space="PSUM") as wt_ps_pool,
    ):
        # ---------------- constants ----------------
        ones_col = scal_pool.tile([P, 1], F32, tag="ones_col")
        nc.vector.memset(ones_col[:], 1.0)
        ones_row = scal_pool.tile([1, P], F32, tag="ones_row")
        nc.vector.memset(ones_row[:], 1.0)
        negmagic = scal_pool.tile([P, 1], F32, tag="negmagic")
        nc.vector.memset(negmagic[:], -MAGIC)
        posmagic = scal_pool.tile([P, 1], F32, tag="posmagic")
        nc.vector.memset(posmagic[:], MAGIC)
        eyef = scal_pool.tile([P, P], F32, tag="eyef")
        nc.sync.dma_start(out=eyef[:], in_=eye_d[:, :])
        idq = scal_pool.tile([P, P], BF16, tag="idq")
        nc.vector.tensor_copy(out=idq[:], in_=eyef[:])

        parts = scal_pool.tile([P, n_wt], F32, tag="parts")

        wf = {}       # j -> resident W f32 tile
        e_st = {}     # i -> per-block state
        deq_t = {}    # i -> dequant scale tile
        clip_ins = {} # j -> wq8 clip-write instruction (for explicit deps)
        pending_deq = []
        sw_state = {}

        # ---------------- emission helpers ----------------
        def emit_w_load(j, pool, name):
            t = pool.tile([P, D], F32, tag="wf32", name=name)
            nc.sync.dma_start(out=t[:], in_=w_d[ts(j, P), :])
            return t

        def emit_absred(j, t):
            # |W| sums split across ACT and DVE (both idle in the load
            # window) so the wre buffers recycle at transfer speed
            if j % 2 == 0:
                scratch = wtmp_pool.tile([P, D], F32, tag="wtA", name=f"absr_{j}")
                nc.scalar.activation(
                    scratch[:], t[:], mybir.ActivationFunctionType.Abs,
                    accum_out=parts[:, j : j + 1],
                )
            else:
                nc.vector.tensor_reduce(
                    out=parts[:, j : j + 1], in_=t[:],
                    axis=mybir.AxisListType.X, op=mybir.AluOpType.add,
                    apply_absolute_value=True,
                )

        def emit_e_load(i):
            ef = ef_pool.tile([P, D], F32, tag="ef", name=f"ef_{i}")
            nc.sync.dma_start(out=ef[:], in_=e_d[ts(i, P), :])
            e_st[i] = {"ef": ef}

        def emit_e_chain(i):
            ef = e_st[i]["ef"]
            rmax = st_pool.tile([P, 1], F32, tag="rmax", name=f"rmax_{i}")
            nc.vector.tensor_reduce(
                out=rmax[:], in_=ef[:], axis=mybir.AxisListType.X,
                op=mybir.AluOpType.max, apply_absolute_value=True,
            )
            rm_c = st_pool.tile([P, 1], F32, tag="rm_c", name=f"rm_c_{i}")
            nc.vector.tensor_scalar_max(rm_c[:], rmax[:], EPS)
            # scale = 127/rm_c with one Newton step on the reciprocal
            r0 = st_pool.tile([P, 1], F32, tag="r0", name=f"r0_{i}")
            nc.vector.reciprocal(r0[:], rm_c[:])
            t1 = st_pool.tile([P, 1], F32, tag="t1s", name=f"t1_{i}")
            nc.vector.scalar_tensor_tensor(
                out=t1[:], in0=rm_c[:], scalar=-1.0, in1=r0[:],
                op0=mybir.AluOpType.mult, op1=mybir.AluOpType.mult,
            )
            nc.vector.tensor_scalar_add(t1[:], t1[:], 2.0)
            nc.vector.tensor_scalar_mul(r0[:], r0[:], t1[:])
            scale = st_pool.tile([P, 1], F32, tag="scale", name=f"scale_{i}")
            nc.vector.tensor_scalar_mul(scale[:], r0[:], 127.0)
            dq = deq_pool.tile([P, 1], F32, tag="deq", name=f"deq_{i}")
            deq_t[i] = (dq, rm_c)
            if "deqm" in sw_state:
                nc.vector.tensor_scalar_mul(dq[:], rm_c[:], sw_state["deqm"][:])
            else:
                # deqm not computed yet: stash rm_c into dq now (st pool
                # buffers recycle quickly), scale by deqm in place later
                nc.vector.tensor_copy(out=dq[:], in_=rm_c[:])
                pending_deq.append(i)
            # single ACT pass: qb = bf16(e*scale); the final fp8 cast in the
            # PSUM evacuation does the (fake-)quant rounding.  Offline-checked
            # rel err 1.498e-2 vs the int8-round reference path's 1.480e-2.
            q8 = q8e_pool.tile([P, D], BF16, tag="q8e", name=f"q8e_{i}")
            nc.scalar.activation(
                q8[:], ef[:], mybir.ActivationFunctionType.Identity,
                scale=scale[:],
            )
            # PE transpose into one PSUM bank (fp8), disjoint chunks one group
            tp = et_ps_pool.tile([P, n_dc, P], BF16, tag="et", name=f"et_{i}")
            for c in range(n_dc):
                nc.tensor.matmul(
                    tp[:, c, :], q8[:, ts(c, P)], idq[:],
                    is_transpose=True, start=(c == 0), stop=(c == n_dc - 1),
                )
            e8 = e8T_pool.tile([P, n_dc, P], FP8, tag="e8T", name=f"e8T_{i}")
            cp = nc.scalar.activation(
                e8[:], tp[:], mybir.ActivationFunctionType.Identity
            )
            e_st[i]["e8T"] = e8
            e_st[i]["copy_ins"] = cp

        def emit_tern_passes(j, src, on_dve=False):
            tA = wtmp_pool.tile([P, D], F32, tag="wtA", name=f"wtA_{j}")
            q8w = wq8pre_pool.tile([P, D], BF16, tag="q8w", name=f"q8w_{j}")
            if on_dve:
                nc.vector.tensor_scalar(
                    out=tA[:], in0=src[:], scalar1=sw_state["s_w"][:],
                    scalar2=MAGIC, op0=mybir.AluOpType.mult,
                    op1=mybir.AluOpType.add,
                )
                nc.vector.tensor_scalar_add(q8w[:], tA[:], -MAGIC)
            else:
                nc.scalar.activation(
                    tA[:], src[:], mybir.ActivationFunctionType.Identity,
                    bias=posmagic[:], scale=sw_state["s_w"][:],
                )
                nc.scalar.activation(
                    q8w[:], tA[:], mybir.ActivationFunctionType.Identity,
                    bias=negmagic[:], scale=1.0,
                )
            return q8w

        def emit_tern_tail(j, q8w):
            tp = wt_ps_pool.tile([P, n_dc, P], BF16, tag="wt", name=f"wt_{j}")
            for c in range(n_dc):
                nc.tensor.matmul(
                    tp[:, c, :], q8w[:, ts(c, P)], idq[:],
                    is_transpose=True, start=(c == 0), stop=(c == n_dc - 1),
                )
            # fused clip(-1,1) + cast into resident transposed fp8 weights
            clip_ins[j] = nc.vector.tensor_scalar(
                out=wq8[:, :, ts(j, P)], in0=tp[:], scalar1=1.0, scalar2=-1.0,
                op0=mybir.AluOpType.min, op1=mybir.AluOpType.max,
            )

        def emit_tern(j, src, on_dve=False):
            emit_tern_tail(j, emit_tern_passes(j, src, on_dve))

        wq8 = wq8_pool.tile([P, n_dc, D], FP8, tag="wq8")

        # ---------------- W pass-1 (uninterrupted on the SP ring) --------
        # Non-resident tiles (j6..12) stream through the 2-buf wre pool
        # FIRST, while nothing else competes and absreds recycle the bufs
        # promptly; residents then burst into wst with no gating.
        n_re = n_wt - N_WST                      # 7 transient tiles
        transient = list(range(6, 6 + n_re))     # j6..j12
        resident = [j for j in range(n_wt) if j not in transient]
        for j in transient:
            t = emit_w_load(j, wre_pool, f"wfm_{j}")
            emit_absred(j, t)
        for j in resident:
            t = emit_w_load(j, wst_pool, f"wfm_{j}")
            wf[j] = t
            emit_absred(j, t)
        # pre-issue the first two W re-loads (wre bufs free after absreds)
        re_tiles = {}
        for j in transient[:2]:
            re_tiles[j] = emit_w_load(j, wre_pool, f"wre_{j}")

        # ---------------- absmean finalize ----------------
        acc = scal_pool.tile([P, 1], F32, tag="acc")
        nc.vector.tensor_reduce(
            out=acc[:], in_=parts[:], axis=mybir.AxisListType.X,
            op=mybir.AluOpType.add,
        )
        # cross-partition sum + broadcast via tiny PE matmuls
        tot_ps = mm_ps_pool.tile([P, MM_N], F32, tag="mm", name="tot_ps")
        nc.tensor.matmul(tot_ps[:1, :1], ones_col[:], acc[:])
        tot_sb = scal_pool.tile([1, 1], F32, tag="tot_sb")
        nc.vector.tensor_copy(out=tot_sb[:], in_=tot_ps[:1, :1])
        asum_ps = mm_ps_pool.tile([P, MM_N], F32, tag="mm", name="asum_ps")
        nc.tensor.matmul(asum_ps[:, :1], ones_row[:], tot_sb[:])
        allsum = scal_pool.tile([P, 1], F32, tag="allsum")
        nc.vector.tensor_copy(out=allsum[:], in_=asum_ps[:, :1])
        # m = max(mean_abs, EPS); s_w = 1/m ; deqm = m/127
        m_t = scal_pool.tile([P, 1], F32, tag="m_t")
        nc.vector.tensor_scalar(
            out=m_t[:], in0=allsum[:], scalar1=1.0 / (D * D), scalar2=EPS,
            op0=mybir.AluOpType.mult, op1=mybir.AluOpType.max,
        )
        r0w = scal_pool.tile([P, 1], F32, tag="r0w")
        nc.vector.reciprocal(r0w[:], m_t[:])
        t1w = scal_pool.tile([P, 1], F32, tag="t1w")
        nc.vector.scalar_tensor_tensor(
            out=t1w[:], in0=m_t[:], scalar=-1.0, in1=r0w[:],
            op0=mybir.AluOpType.mult, op1=mybir.AluOpType.mult,
        )
        nc.vector.tensor_scalar_add(t1w[:], t1w[:], 2.0)
        s_w = scal_pool.tile([P, 1], F32, tag="s_w")
        nc.vector.tensor_scalar_mul(s_w[:], r0w[:], t1w[:])
        deqm = scal_pool.tile([P, 1], F32, tag="deqm")
        nc.vector.tensor_scalar_mul(deqm[:], m_t[:], 1.0 / 127.0)
        sw_state["s_w"] = s_w
        sw_state["deqm"] = deqm
        for i in pending_deq:
            dq, _ = deq_t[i]
            nc.vector.tensor_scalar_mul(dq[:], dq[:], deqm[:])
        pending_deq.clear()

        # ---------------- A_eff (only if nonzero A_raw) ----------------
        if with_h:
            a_d = io["a_raw"]
            a1 = scal_pool.tile([1, D], F32, tag="a1")
            nc.sync.dma_start(out=a1[:], in_=a_d[:, :])
            aeff = scal_pool.tile([P, D], F32, tag="aeff")
            for ob in range(n_ob):
                ab_ps = mm_ps_pool.tile([P, MM_N], F32, tag="mm", name=f"ab_{ob}")
                nc.tensor.matmul(ab_ps[:], ones_row[:], a1[:, ts(ob, MM_N)])
                nc.vector.tensor_copy(out=aeff[:, ts(ob, MM_N)], in_=ab_ps[:])
            nc.scalar.activation(
                aeff[:], aeff[:], mybir.ActivationFunctionType.Tanh
            )
            nc.vector.tensor_scalar_mul(aeff[:], aeff[:], 0.99)

        # ---------------- ternarize head (j0..3 unblocks ob=0) ----------
        # j0/j2 round on ACT while j1/j3 round on DVE; tails in j order
        head_q8w = {}
        for j in range(4):
            head_q8w[j] = emit_tern_passes(j, wf[j], on_dve=(j % 2 == 1))
        for j in range(4):
            emit_tern_tail(j, head_q8w[j])

        # ---------------- main loop: (ob, i) units, ob-major w/ lag -----
        LAG = 5
        unit_order = []
        for i in range(n_tb):
            unit_order.append((0, i))
            if i >= LAG:
                unit_order.append((1, i - LAG))
        unit_order += [(1, i) for i in range(n_tb - LAG, n_tb)]
        for ob in range(2, n_ob):
            for i in range(n_tb):
                unit_order.append((ob, i))

        bo_t = {}

        def emit_bo_load(ob, i):
            t = bo_pool.tile([P, MM_N], F32, tag="bo", name=f"bo_{ob}_{i}")
            nc.gpsimd.dma_start(
                out=t[:], in_=bo_d[ts(i, P), ts(ob, MM_N)]
            )
            bo_t[(ob, i)] = t

        def emit_tern_task(j):
            if j in wf:
                src = wf[j]
            elif j in re_tiles:
                src = re_tiles.pop(j)
            else:
                src = emit_w_load(j, wre_pool, f"wre_{j}")
            emit_tern(j, src)

        def emit_e_task(i):
            emit_e_load(i)
            emit_e_chain(i)

        # e0..e3 must exist before the unit loop touches them
        for i in range(4):
            emit_e_task(i)

        # side tasks in need-time order (one per unit): e(i) before unit
        # (0,i); terns j4..7 before the ob=1 sweep (unit idx 6); terns
        # j8..15 before the ob=2/3 sweeps (unit idx 32/48)
        side_list = (
            [lambda i=i: emit_e_task(i) for i in range(4, 6)]
            + [lambda j=j: emit_tern_task(j) for j in range(4, 8)]
            + [lambda i=i: emit_e_task(i) for i in range(6, n_tb)]
            + [lambda j=j: emit_tern_task(j) for j in range(8, n_wt)]
        )
        side_pos = [0]

        def emit_side_task():
            if side_pos[0] < len(side_list):
                side_list[side_pos[0]]()
                side_pos[0] += 1

        seen_ob = set()
        seen_e8 = set()

        def emit_mm_unit(ob, i):
            e8 = e_st[i]["e8T"]
            ps = mm_ps_pool.tile([P, MM_N], F32, tag="mm", name=f"mm_{ob}_{i}")
            for c in range(n_pr):
                mm = nc.tensor.matmul(
                    ps[:],
                    e8[:, 2 * c : 2 * c + 2, :],
                    wq8[:, 2 * c : 2 * c + 2, ts(ob, MM_N)],
                    start=(c == 0),
                    stop=(c == n_pr - 1),
                    perf_mode=DR,
                )
                if c == 0:
                    # Tile's range analysis misses RAW deps through these
                    # strided 3D APs (verified on hw: reads raced the wq8
                    # clips).  Pin them explicitly; PE is in-order so only
                    # the first consumer needs each edge.
                    if ob not in seen_ob:
                        seen_ob.add(ob)
                        for j in range(4 * ob, 4 * ob + 4):
                            add_dep_helper(
                                mm.ins, clip_ins[j].ins, sync=True,
                                reason=f"mm(ob{ob}) after wq8 clip j{j}",
                            )
                    if i not in seen_e8:
                        seen_e8.add(i)
                        add_dep_helper(
                            mm.ins, e_st[i]["copy_ins"].ins, sync=True,
                            reason=f"mm after e8T copy {i}",
                        )
            bt = bo_t.pop((ob, i))
            # bt = psum * deq + block_out   (fused dequant + add, in place)
            nc.vector.scalar_tensor_tensor(
                out=bt[:], in0=ps[:], scalar=deq_t[i][0][:], in1=bt[:],
                op0=mybir.AluOpType.mult, op1=mybir.AluOpType.add,
            )
            if with_h:
                hq = bo_pool.tile([P, MM_N], F32, tag="bo", name=f"h_{ob}_{i}")
                nc.gpsimd.dma_start(
                    out=hq[:], in_=io["h"][ts(i, P), ts(ob, MM_N)]
                )
                nc.vector.tensor_tensor(
                    out=hq[:], in0=hq[:], in1=aeff[:, ts(ob, MM_N)],
                    op=mybir.AluOpType.mult,
                )
                nc.vector.tensor_tensor(
                    out=bt[:], in0=bt[:], in1=hq[:], op=mybir.AluOpType.add,
                )
            nc.gpsimd.dma_start(
                out=out_d[ts(i, P), ts(ob, MM_N)], in_=bt[:]
            )

        for k in range(min(BO_PRE, len(unit_order))):
            emit_bo_load(*unit_order[k])
        for k, (ob, i) in enumerate(unit_order):
            emit_side_task()
            if k + BO_PRE < len(unit_order):
                emit_bo_load(*unit_order[k + BO_PRE])
            emit_mm_unit(ob, i)
        # drain any leftover side work (shouldn't happen, but be safe)
        while side_pos[0] < len(side_list):
            emit_side_task()


def legalize_waits(nc):
    """Walrus in this container encodes at most ONE sync wait per ISA
    instruction (the 64B Events field) and refuses to split.  Rewrite any
    instruction carrying N>1 waits into N-1 single-wait NOP carrier
    instructions on the same engine placed immediately before it, keeping one
    wait on the original.  Waits are monotonic sem>=v conditions, so splitting
    preserves semantics exactly."""
    import bass_rust

    eng_map = {
        mybir.EngineType.SP: nc.sync,
        mybir.EngineType.DVE: nc.vector,
        mybir.EngineType.Activation: nc.scalar,
        mybir.EngineType.PE: nc.tensor,
        mybir.EngineType.Pool: nc.gpsimd,
    }
    for f in nc.m.functions:
        for blk in f.blocks:
            insts = list(blk.instructions)
            if not any(
                i.sync_info is not None and len(i.sync_info.on_wait) > 1
                for i in insts
            ):
                continue
            carriers = {}  # target inst name -> list of carrier insts
            for inst in insts:
                si = inst.sync_info
                if si is None or len(si.on_wait) <= 1:
                    continue
                waits = list(si.on_wait)
                cs = []
                for w in waits[:-1]:
                    bi = eng_map[inst.engine].nop(nofuse=True)
                    nop_inst = bi.ins
                    nop_inst.sync_info = bass_rust.SyncInfo(
                        on_wait=[w], on_update=[]
                    )
                    cs.append(nop_inst)
                carriers[inst.name] = cs
                inst.sync_info = bass_rust.SyncInfo(
                    on_wait=[waits[-1]], on_update=list(si.on_update)
                )
            # nops were appended to the current bb; remove them from wherever
            # they landed and splice before their targets.
            carrier_names = {c.name for cs in carriers.values() for c in cs}
            for f2 in nc.m.functions:
                for blk2 in f2.blocks:
                    cur = list(blk2.instructions)
                    if any(i.name in carrier_names for i in cur):
                        blk2.instructions = [
                            i for i in cur if i.name not in carrier_names
                        ]
            new_list = []
            for inst in blk.instructions:
                for c in carriers.get(inst.name, ()):
                    new_list.append(c)
                new_list.append(inst)
            blk.instructions = new_list


def build_nc(Tc: int, D: int, with_h: bool):
    nc = bass.Bass("TRN2", target_bir_lowering=False, debug=False)
    io = {
        "e": nc.declare_dram_parameter("e", [Tc, D], F32, isOutput=False)[:],
        "bo": nc.declare_dram_parameter("bo", [Tc, D], F32, isOutput=False)[:],
        "w": nc.declare_dram_parameter("w", [D, D], F32, isOutput=False)[:],
        "eye": nc.declare_dram_parameter("eye", [P, P], F32, isOutput=False)[:],
    }
    if with_h:
        io["h"] = nc.declare_dram_parameter("h", [Tc, D], F32, isOutput=False)[:]
        io["a_raw"] = nc.declare_dram_parameter("a_raw", [1, D], F32, isOutput=False)[:]
    io["out"] = nc.declare_dram_parameter("out", [Tc, D], F32, isOutput=True)[:]
    with tile.TileContext(nc) as tc:
        build_kernel_body(tc, io, Tc, D, with_h)
    legalize_waits(nc)
    return nc


_NC_CACHE: dict = {}


def _get_nc(Tc: int, D: int, with_h: bool):
    key = (Tc, D, with_h)
    if key not in _NC_CACHE:
        _NC_CACHE[key] = build_nc(Tc, D, with_h)
    return _NC_CACHE[key]


def kernel(h, e, block_out, A_raw, W, _trace=False, _trace_kwargs=None):
    Bb, Tt, D = e.shape
    rows = Bb * Tt
    Tc = rows // N_CORES
    e2 = e.reshape(rows, D)
    bo2 = block_out.reshape(rows, D)
    h2 = h.reshape(rows, D)
    with_h = bool(np.any(A_raw))
    eye = np.eye(P, dtype=np.float32)

    nc = _get_nc(Tc, D, with_h)
    in_maps = []
    for c in range(N_CORES):
        sl = slice(c * Tc, (c + 1) * Tc)
        m = {
            "e": np.ascontiguousarray(e2[sl]),
            "bo": np.ascontiguousarray(bo2[sl]),
            "w": np.ascontiguousarray(W),
            "eye": eye,
        }
        if with_h:
            m["h"] = np.ascontiguousarray(h2[sl])
            m["a_raw"] = np.ascontiguousarray(A_raw.reshape(1, D))
        in_maps.append(m)

    res = run_bass_kernel_spmd(
        nc, in_maps, list(range(N_CORES)), trace=_trace,
        **(_trace_kwargs or {}),
    )
    out = np.concatenate([res.results[c]["out"] for c in range(N_CORES)], axis=0)
    if _trace:
        return out.reshape(Bb, Tt, D), res
    return out.reshape(Bb, Tt, D)


# revision 22
# speedup vs baseline: 1.0374x; 1.0374x over previous
"""Trainium2 Bass kernel for BitLTIInjection (BitNet-style fake-quantized linear
+ LTI injection):

    A_eff = 0.99*tanh(A_raw)
    e_q   = per-token absmax int8 fake quant of e
    W_q   = absmean ternary fake quant of W
    out   = A_eff*h + e_q @ W_q.T + block_out

Strategy v2: data-parallel over B*T across 8 cores; W replicated.

The quantized matmul runs in fp8e4 with MatmulPerfMode.DoubleRow (K=256 per
matmul, 2x MAC rate).  W_q in {-1,0,1} is exact in e4m3; e_q (ints in
[-128,127]) is RNE-cast to e4m3 which introduces a small, offline-verified
error (rel err ~1.5e-2 < 2e-2 gate).  Rounding uses the f32 magic-number
trick (x + 1.5*2^23 - 1.5*2^23 = RNE-to-integer).

Schedule: all transposes run on the PE (fp8 transpose-mode into PSUM, with
the clip/cast fused into the PSUM->SBUF DVE evacuation), keeping the DMA
fabric for pure HBM streaming.  The main loop is ob-major over (token-block,
out-column-block) units of [128x512] so matmuls start as soon as the first 4
W column-tiles are ternarized (~65us) instead of after full W prep.  9 of 16
W f32 tiles stay resident in SBUF after the absmean pass; 7 are re-loaded.
"""

import numpy as np

import concourse.bass as bass
import concourse.mybir as mybir
import concourse.tile as tile
from concourse.tile_rust import add_dep_helper
from concourse.bass import ts
from concourse.bass_utils import run_bass_kernel_spmd

P = 128
MAGIC = 12582912.0  # 1.5 * 2**23: forces RNE-to-integer in f32
EPS = 1e-5
N_CORES = 8
F32 = mybir.dt.float32
BF16 = mybir.dt.bfloat16
FP8 = mybir.dt.float8e4
MM_N = 512   # psum tile free dim (one f32 bank)
N_WST = 9    # W f32 tiles kept resident after absmean pass
BO_PRE = 5   # block_out quarter-tile prefetch depth (units)
DR = mybir.MatmulPerfMode.DoubleRow


def build_kernel_body(tc: tile.TileContext, io: dict, Tc: int, D: int, with_h: bool):
    nc = tc.nc
    n_tb = Tc // P     # token blocks per core
    n_dc = D // P      # contraction chunks of 128
    n_ob = D // MM_N   # output column blocks
    n_wt = D // P      # weight row tiles (j)
    n_pr = n_dc // 2   # DoubleRow K-pairs

    e_d = io["e"]
    bo_d = io["bo"]
    w_d = io["w"]
    eye_d = io["eye"]
    out_d = io["out"]

    with (
        tc.tile_pool(name="wst", bufs=N_WST) as wst_pool,
        tc.tile_pool(name="wre", bufs=8) as wre_pool,
        tc.tile_pool(name="wq8", bufs=1) as wq8_pool,
        tc.tile_pool(name="wtA", bufs=2) as wtA_pool,
        tc.tile_pool(name="wq8pre", bufs=2) as wq8pre_pool,
        tc.tile_pool(name="ef", bufs=2) as ef_pool,
        tc.tile_pool(name="q8e", bufs=2) as q8e_pool,
        tc.tile_pool(name="e8T", bufs=n_tb) as e8T_pool,
        tc.tile_pool(name="bo", bufs=6) as bo_pool,
        tc.tile_pool(name="scal", bufs=1) as scal_pool,
        tc.tile_pool(name="st", bufs=3) as st_pool,
        tc.tile_pool(name="deq", bufs=n_tb) as deq_pool,
        tc.tile_pool(name="mm_ps", bufs=2, space="PSUM") as mm_ps_pool,
        tc.tile_pool(name="et_ps", bufs=2, space="PSUM") as et_ps_pool,
        tc.tile_pool(name="wt_ps", bufs=1, space="PSUM") as wt_ps_pool,
    ):
        # ---------------- constants ----------------
        ones_col = scal_pool.tile([P, 1], F32, tag="ones_col")
        nc.vector.memset(ones_col[:], 1.0)
        ones_row = scal_pool.tile([1, P], F32, tag="ones_row")
        nc.vector.memset(ones_row[:], 1.0)
        negmagic = scal_pool.tile([P, 1], F32, tag="negmagic")
        nc.vector.memset(negmagic[:], -MAGIC)
        posmagic = scal_pool.tile([P, 1], F32, tag="posmagic")
        nc.vector.memset(posmagic[:], MAGIC)
        eyef = scal_pool.tile([P, P], F32, tag="eyef")
        nc.sync.dma_start(out=eyef[:], in_=eye_d[:, :])
        idq = scal_pool.tile([P, P], BF16, tag="idq")
        nc.vector.tensor_copy(out=idq[:], in_=eyef[:])

        parts = scal_pool.tile([P, 4 * n_wt], F32, tag="parts")
        nc.vector.memset(parts[:], 0.0)

        wf = {}       # j -> resident W f32 tile
        e_st = {}     # i -> per-block state
        deq_t = {}    # i -> dequant scale tile
        clip_ins = {} # j -> wq8 clip-write instruction (for explicit deps)
        pending_deq = []
        sw_state = {}

        # ---------------- emission helpers ----------------
        absred_alt = [0]

        def emit_w_quarter_load(j, q, name):
            t = wre_pool.tile([P, MM_N], F32, tag="wq32", name=name)
            nc.sync.dma_start(out=t[:], in_=w_d[ts(j, P), ts(q, MM_N)])
            return t

        def emit_absred_q(j, q, src_ap):
            # per-quarter |W| sums, alternating ACT/DVE (both idle in the
            # load window) so wre buffers recycle at transfer speed
            col = 4 * j + q
            absred_alt[0] ^= 1
            if absred_alt[0]:
                scratch = wtA_pool.tile(
                    [P, MM_N], F32, tag="wtA", name=f"absr_{j}_{q}"
                )
                nc.scalar.activation(
                    scratch[:], src_ap, mybir.ActivationFunctionType.Abs,
                    accum_out=parts[:, col : col + 1],
                )
            else:
                nc.vector.tensor_reduce(
                    out=parts[:, col : col + 1], in_=src_ap,
                    axis=mybir.AxisListType.X, op=mybir.AluOpType.add,
                    apply_absolute_value=True,
                )

        def emit_e_load(i):
            ef = ef_pool.tile([P, D], F32, tag="ef", name=f"ef_{i}")
            nc.sync.dma_start(out=ef[:], in_=e_d[ts(i, P), :])
            e_st[i] = {"ef": ef}

        def emit_e_chain(i):
            ef = e_st[i]["ef"]
            rmax = st_pool.tile([P, 1], F32, tag="rmax", name=f"rmax_{i}")
            nc.vector.tensor_reduce(
                out=rmax[:], in_=ef[:], axis=mybir.AxisListType.X,
                op=mybir.AluOpType.max, apply_absolute_value=True,
            )
            rm_c = st_pool.tile([P, 1], F32, tag="rm_c", name=f"rm_c_{i}")
            nc.vector.tensor_scalar_max(rm_c[:], rmax[:], EPS)
            # scale = 127/rm_c with one Newton step on the reciprocal
            r0 = st_pool.tile([P, 1], F32, tag="r0", name=f"r0_{i}")
            nc.vector.reciprocal(r0[:], rm_c[:])
            t1 = st_pool.tile([P, 1], F32, tag="t1s", name=f"t1_{i}")
            nc.vector.scalar_tensor_tensor(
                out=t1[:], in0=rm_c[:], scalar=-1.0, in1=r0[:],
                op0=mybir.AluOpType.mult, op1=mybir.AluOpType.mult,
            )
            nc.vector.tensor_scalar_add(t1[:], t1[:], 2.0)
            nc.vector.tensor_scalar_mul(r0[:], r0[:], t1[:])
            scale = st_pool.tile([P, 1], F32, tag="scale", name=f"scale_{i}")
            nc.vector.tensor_scalar_mul(scale[:], r0[:], 127.0)
            dq = deq_pool.tile([P, 1], F32, tag="deq", name=f"deq_{i}")
            deq_t[i] = (dq, rm_c)
            if "deqm" in sw_state:
                nc.vector.tensor_scalar_mul(dq[:], rm_c[:], sw_state["deqm"][:])
            else:
                # deqm not computed yet: stash rm_c into dq now (st pool
                # buffers recycle quickly), scale by deqm in place later
                nc.vector.tensor_copy(out=dq[:], in_=rm_c[:])
                pending_deq.append(i)
            # single ACT pass: qb = bf16(e*scale); the final fp8 cast in the
            # PSUM evacuation does the (fake-)quant rounding.  Offline-checked
            # rel err 1.498e-2 vs the int8-round reference path's 1.480e-2.
            q8 = q8e_pool.tile([P, D], BF16, tag="q8e", name=f"q8e_{i}")
            nc.scalar.activation(
                q8[:], ef[:], mybir.ActivationFunctionType.Identity,
                scale=scale[:],
            )
            # PE transpose into one PSUM bank (fp8), disjoint chunks one group
            tp = et_ps_pool.tile([P, n_dc, P], BF16, tag="et", name=f"et_{i}")
            for c in range(n_dc):
                nc.tensor.matmul(
                    tp[:, c, :], q8[:, ts(c, P)], idq[:],
                    is_transpose=True, start=(c == 0), stop=(c == n_dc - 1),
                )
            e8 = e8T_pool.tile([P, n_dc, P], FP8, tag="e8T", name=f"e8T_{i}")
            cp = nc.scalar.activation(
                e8[:], tp[:], mybir.ActivationFunctionType.Identity
            )
            e_st[i]["e8T"] = e8
            e_st[i]["copy_ins"] = cp

        def emit_tern_quarter(j, q, src_ap, tp, on_dve=False):
            tA = wtA_pool.tile([P, MM_N], F32, tag="wtA", name=f"wtA_{j}_{q}")
            q8w = wq8pre_pool.tile(
                [P, MM_N], BF16, tag="q8w", name=f"q8w_{j}_{q}"
            )
            if on_dve:
                nc.vector.tensor_scalar(
                    out=tA[:], in0=src_ap, scalar1=sw_state["s_w"][:],
                    scalar2=MAGIC, op0=mybir.AluOpType.mult,
                    op1=mybir.AluOpType.add,
                )
                nc.vector.tensor_scalar_add(q8w[:], tA[:], -MAGIC)
            else:
                nc.scalar.activation(
                    tA[:], src_ap, mybir.ActivationFunctionType.Identity,
                    bias=posmagic[:], scale=sw_state["s_w"][:],
                )
                nc.scalar.activation(
                    q8w[:], tA[:], mybir.ActivationFunctionType.Identity,
                    bias=negmagic[:], scale=1.0,
                )
            n_qc = MM_N // P
            for c in range(n_qc):
                nc.tensor.matmul(
                    tp[:, n_qc * q + c, :], q8w[:, ts(c, P)], idq[:],
                    is_transpose=True,
                    start=(q == 0 and c == 0),
                    stop=(q == n_ob - 1 and c == n_qc - 1),
                )

        def emit_tern(j, srcs, on_dve=False):
            # srcs: list of n_ob quarter APs ([P, MM_N] f32)
            tp = wt_ps_pool.tile([P, n_dc, P], BF16, tag="wt", name=f"wt_{j}")
            for q in range(n_ob):
                emit_tern_quarter(j, q, srcs[q], tp, on_dve)
            # fused clip(-1,1) + cast into resident transposed fp8 weights
            clip_ins[j] = nc.vector.tensor_scalar(
                out=wq8[:, :, ts(j, P)], in0=tp[:], scalar1=1.0, scalar2=-1.0,
                op0=mybir.AluOpType.min, op1=mybir.AluOpType.max,
            )

        wq8 = wq8_pool.tile([P, n_dc, D], FP8, tag="wq8")

        # ---------------- W pass-1 --------------------------------------
        # Residents (j0..5, j13..15) burst into wst with no gating; the
        # transient tiles (j6..12) then stream as quarter tiles through the
        # 8-deep wre pool, so recycling is throughput- not latency-bound.
        n_re = n_wt - N_WST                      # 7 transient tiles
        transient = list(range(6, 6 + n_re))     # j6..j12
        resident = [j for j in range(n_wt) if j not in transient]
        for j in resident:
            t = wst_pool.tile([P, D], F32, tag="wf32", name=f"wfm_{j}")
            nc.sync.dma_start(out=t[:], in_=w_d[ts(j, P), :])
            wf[j] = t
            for q in range(n_ob):
                emit_absred_q(j, q, t[:, ts(q, MM_N)])
        for j in transient:
            for q in range(n_ob):
                t = emit_w_quarter_load(j, q, f"wfm_{j}_{q}")
                emit_absred_q(j, q, t[:])
        # pre-issue re-load quarters for the first two transient tiles
        re_tiles = {}
        for j in transient[:2]:
            re_tiles[j] = [
                emit_w_quarter_load(j, q, f"wre_{j}_{q}") for q in range(n_ob)
            ]

        # ---------------- absmean finalize ----------------
        acc = scal_pool.tile([P, 1], F32, tag="acc")
        nc.vector.tensor_reduce(
            out=acc[:], in_=parts[:], axis=mybir.AxisListType.X,
            op=mybir.AluOpType.add,
        )
        # cross-partition sum + broadcast via tiny PE matmuls
        tot_ps = mm_ps_pool.tile([P, MM_N], F32, tag="mm", name="tot_ps")
        nc.tensor.matmul(tot_ps[:1, :1], ones_col[:], acc[:])
        tot_sb = scal_pool.tile([1, 1], F32, tag="tot_sb")
        nc.vector.tensor_copy(out=tot_sb[:], in_=tot_ps[:1, :1])
        asum_ps = mm_ps_pool.tile([P, MM_N], F32, tag="mm", name="asum_ps")
        nc.tensor.matmul(asum_ps[:, :1], ones_row[:], tot_sb[:])
        allsum = scal_pool.tile([P, 1], F32, tag="allsum")
        nc.vector.tensor_copy(out=allsum[:], in_=asum_ps[:, :1])
        # m = max(mean_abs, EPS); s_w = 1/m ; deqm = m/127
        m_t = scal_pool.tile([P, 1], F32, tag="m_t")
        nc.vector.tensor_scalar(
            out=m_t[:], in0=allsum[:], scalar1=1.0 / (D * D), scalar2=EPS,
            op0=mybir.AluOpType.mult, op1=mybir.AluOpType.max,
        )
        r0w = scal_pool.tile([P, 1], F32, tag="r0w")
        nc.vector.reciprocal(r0w[:], m_t[:])
        t1w = scal_pool.tile([P, 1], F32, tag="t1w")
        nc.vector.scalar_tensor_tensor(
            out=t1w[:], in0=m_t[:], scalar=-1.0, in1=r0w[:],
            op0=mybir.AluOpType.mult, op1=mybir.AluOpType.mult,
        )
        nc.vector.tensor_scalar_add(t1w[:], t1w[:], 2.0)
        s_w = scal_pool.tile([P, 1], F32, tag="s_w")
        nc.vector.tensor_scalar_mul(s_w[:], r0w[:], t1w[:])
        deqm = scal_pool.tile([P, 1], F32, tag="deqm")
        nc.vector.tensor_scalar_mul(deqm[:], m_t[:], 1.0 / 127.0)
        sw_state["s_w"] = s_w
        sw_state["deqm"] = deqm
        for i in pending_deq:
            dq, _ = deq_t[i]
            nc.vector.tensor_scalar_mul(dq[:], dq[:], deqm[:])
        pending_deq.clear()

        # ---------------- A_eff (only if nonzero A_raw) ----------------
        if with_h:
            a_d = io["a_raw"]
            a1 = scal_pool.tile([1, D], F32, tag="a1")
            nc.sync.dma_start(out=a1[:], in_=a_d[:, :])
            aeff = scal_pool.tile([P, D], F32, tag="aeff")
            for ob in range(n_ob):
                ab_ps = mm_ps_pool.tile([P, MM_N], F32, tag="mm", name=f"ab_{ob}")
                nc.tensor.matmul(ab_ps[:], ones_row[:], a1[:, ts(ob, MM_N)])
                nc.vector.tensor_copy(out=aeff[:, ts(ob, MM_N)], in_=ab_ps[:])
            nc.scalar.activation(
                aeff[:], aeff[:], mybir.ActivationFunctionType.Tanh
            )
            nc.vector.tensor_scalar_mul(aeff[:], aeff[:], 0.99)

        # ---------------- ternarize head (j0..3 unblocks ob=0) ----------
        # j0/j2 round on ACT while j1/j3 round on DVE
        for j in range(4):
            emit_tern(
                j, [wf[j][:, ts(q, MM_N)] for q in range(n_ob)],
                on_dve=(j % 2 == 1),
            )

        # ---------------- main loop: (ob, i) units, ob-major w/ lag -----
        LAG = 5
        unit_order = []
        for i in range(n_tb):
            unit_order.append((0, i))
            if i >= LAG:
                unit_order.append((1, i - LAG))
        unit_order += [(1, i) for i in range(n_tb - LAG, n_tb)]
        for ob in range(2, n_ob):
            for i in range(n_tb):
                unit_order.append((ob, i))

        bo_t = {}

        def emit_bo_load(ob, i):
            t = bo_pool.tile([P, MM_N], F32, tag="bo", name=f"bo_{ob}_{i}")
            nc.gpsimd.dma_start(
                out=t[:], in_=bo_d[ts(i, P), ts(ob, MM_N)]
            )
            bo_t[(ob, i)] = t

        def emit_tern_task(j):
            if j in wf:
                srcs = [wf[j][:, ts(q, MM_N)] for q in range(n_ob)]
            elif j in re_tiles:
                srcs = [t[:] for t in re_tiles.pop(j)]
            else:
                srcs = [
                    emit_w_quarter_load(j, q, f"wre_{j}_{q}")[:]
                    for q in range(n_ob)
                ]
            emit_tern(j, srcs)

        def emit_e_task(i):
            emit_e_load(i)
            emit_e_chain(i)

        # e0..e3 must exist before the unit loop touches them
        for i in range(4):
            emit_e_task(i)

        # side tasks in need-time order (one per unit): e(i) before unit
        # (0,i); terns j4..7 before the ob=1 sweep (unit idx 6); terns
        # j8..15 before the ob=2/3 sweeps (unit idx 32/48)
        side_list = (
            [lambda i=i: emit_e_task(i) for i in range(4, 6)]
            + [lambda j=j: emit_tern_task(j) for j in range(4, 8)]
            + [lambda i=i: emit_e_task(i) for i in range(6, n_tb)]
            + [lambda j=j: emit_tern_task(j) for j in range(8, n_wt)]
        )
        side_pos = [0]

        def emit_side_task():
            if side_pos[0] < len(side_list):
                side_list[side_pos[0]]()
                side_pos[0] += 1

        seen_ob = set()
        seen_e8 = set()

        def emit_mm_unit(ob, i):
            e8 = e_st[i]["e8T"]
            ps = mm_ps_pool.tile([P, MM_N], F32, tag="mm", name=f"mm_{ob}_{i}")
            for c in range(n_pr):
                mm = nc.tensor.matmul(
                    ps[:],
                    e8[:, 2 * c : 2 * c + 2, :],
                    wq8[:, 2 * c : 2 * c + 2, ts(ob, MM_N)],
                    start=(c == 0),
                    stop=(c == n_pr - 1),
                    perf_mode=DR,
                )
                if c == 0:
                    # Tile's range analysis misses RAW deps through these
                    # strided 3D APs (verified on hw: reads raced the wq8
                    # clips).  Pin them explicitly; PE is in-order so only
                    # the first consumer needs each edge.
                    if ob not in seen_ob:
                        seen_ob.add(ob)
                        for j in range(4 * ob, 4 * ob + 4):
                            add_dep_helper(
                                mm.ins, clip_ins[j].ins, sync=True,
                                reason=f"mm(ob{ob}) after wq8 clip j{j}",
                            )
                    if i not in seen_e8:
                        seen_e8.add(i)
                        add_dep_helper(
                            mm.ins, e_st[i]["copy_ins"].ins, sync=True,
                            reason=f"mm after e8T copy {i}",
                        )
            bt = bo_t.pop((ob, i))
            # bt = psum * deq + block_out   (fused dequant + add, in place)
            nc.vector.scalar_tensor_tensor(
                out=bt[:], in0=ps[:], scalar=deq_t[i][0][:], in1=bt[:],
                op0=mybir.AluOpType.mult, op1=mybir.AluOpType.add,
            )
            if with_h:
                hq = bo_pool.tile([P, MM_N], F32, tag="bo", name=f"h_{ob}_{i}")
                nc.gpsimd.dma_start(
                    out=hq[:], in_=io["h"][ts(i, P), ts(ob, MM_N)]
                )
                nc.vector.tensor_tensor(
                    out=hq[:], in0=hq[:], in1=aeff[:, ts(ob, MM_N)],
                    op=mybir.AluOpType.mult,
                )
                nc.vector.tensor_tensor(
                    out=bt[:], in0=bt[:], in1=hq[:], op=mybir.AluOpType.add,
                )
            nc.gpsimd.dma_start(
                out=out_d[ts(i, P), ts(ob, MM_N)], in_=bt[:]
            )

        for k in range(min(BO_PRE, len(unit_order))):
            emit_bo_load(*unit_order[k])
        for k, (ob, i) in enumerate(unit_order):
            emit_side_task()
            if k + BO_PRE < len(unit_order):
                emit_bo_load(*unit_order[k + BO_PRE])
            emit_mm_unit(ob, i)
        # drain any leftover side work (shouldn't happen, but be safe)
        while side_pos[0] < len(side_list):
            emit_side_task()


def legalize_waits(nc):
    """Walrus in this container encodes at most ONE sync wait per ISA
    instruction (the 64B Events field) and refuses to split.  Rewrite any
    instruction carrying N>1 waits into N-1 single-wait NOP carrier
    instructions on the same engine placed immediately before it, keeping one
    wait on the original.  Waits are monotonic sem>=v conditions, so splitting
    preserves semantics exactly."""
    import bass_rust

    eng_map = {
        mybir.EngineType.SP: nc.sync,
        mybir.EngineType.DVE: nc.vector,
        mybir.EngineType.Activation: nc.scalar,
        mybir.EngineType.PE: nc.tensor,
        mybir.EngineType.Pool: nc.gpsimd,
    }
    for f in nc.m.functions:
        for blk in f.blocks:
            insts = list(blk.instructions)
            if not any(
                i.sync_info is not None and len(i.sync_info.on_wait) > 1
                for i in insts
            ):
                continue
            carriers = {}  # target inst name -> list of carrier insts
            for inst in insts:
                si = inst.sync_info
                if si is None or len(si.on_wait) <= 1:
                    continue
                waits = list(si.on_wait)
                cs = []
                for w in waits[:-1]:
                    bi = eng_map[inst.engine].nop(nofuse=True)
                    nop_inst = bi.ins
                    nop_inst.sync_info = bass_rust.SyncInfo(
                        on_wait=[w], on_update=[]
                    )
                    cs.append(nop_inst)
                carriers[inst.name] = cs
                inst.sync_info = bass_rust.SyncInfo(
                    on_wait=[waits[-1]], on_update=list(si.on_update)
                )
            # nops were appended to the current bb; remove them from wherever
            # they landed and splice before their targets.
            carrier_names = {c.name for cs in carriers.values() for c in cs}
            for f2 in nc.m.functions:
                for blk2 in f2.blocks:
                    cur = list(blk2.instructions)
                    if any(i.name in carrier_names for i in cur):
                        blk2.instructions = [
                            i for i in cur if i.name not in carrier_names
                        ]
            new_list = []
            for inst in blk.instructions:
                for c in carriers.get(inst.name, ()):
                    new_list.append(c)
                new_list.append(inst)
            blk.instructions = new_list


def build_nc(Tc: int, D: int, with_h: bool):
    nc = bass.Bass("TRN2", target_bir_lowering=False, debug=False)
    io = {
        "e": nc.declare_dram_parameter("e", [Tc, D], F32, isOutput=False)[:],
        "bo": nc.declare_dram_parameter("bo", [Tc, D], F32, isOutput=False)[:],
        "w": nc.declare_dram_parameter("w", [D, D], F32, isOutput=False)[:],
        "eye": nc.declare_dram_parameter("eye", [P, P], F32, isOutput=False)[:],
    }
    if with_h:
        io["h"] = nc.declare_dram_parameter("h", [Tc, D], F32, isOutput=False)[:]
        io["a_raw"] = nc.declare_dram_parameter("a_raw", [1, D], F32, isOutput=False)[:]
    io["out"] = nc.declare_dram_parameter("out", [Tc, D], F32, isOutput=True)[:]
    with tile.TileContext(nc) as tc:
        build_kernel_body(tc, io, Tc, D, with_h)
    legalize_waits(nc)
    return nc


_NC_CACHE: dict = {}


def _get_nc(Tc: int, D: int, with_h: bool):
    key = (Tc, D, with_h)
    if key not in _NC_CACHE:
        _NC_CACHE[key] = build_nc(Tc, D, with_h)
    return _NC_CACHE[key]


def kernel(h, e, block_out, A_raw, W, _trace=False, _trace_kwargs=None):
    Bb, Tt, D = e.shape
    rows = Bb * Tt
    Tc = rows // N_CORES
    e2 = e.reshape(rows, D)
    bo2 = block_out.reshape(rows, D)
    h2 = h.reshape(rows, D)
    with_h = bool(np.any(A_raw))
    eye = np.eye(P, dtype=np.float32)

    nc = _get_nc(Tc, D, with_h)
    in_maps = []
    for c in range(N_CORES):
        sl = slice(c * Tc, (c + 1) * Tc)
        m = {
            "e": np.ascontiguousarray(e2[sl]),
            "bo": np.ascontiguousarray(bo2[sl]),
            "w": np.ascontiguousarray(W),
            "eye": eye,
        }
        if with_h:
            m["h"] = np.ascontiguousarray(h2[sl])
            m["a_raw"] = np.ascontiguousarray(A_raw.reshape(1, D))
        in_maps.append(m)

    res = run_bass_kernel_spmd(
        nc, in_maps, list(range(N_CORES)), trace=_trace,
        **(_trace_kwargs or {}),
    )
    out = np.concatenate([res.results[c]["out"] for c in range(N_CORES)], axis=0)
    if _trace:
        return out.reshape(Bb, Tt, D), res
    return out.reshape(Bb, Tt, D)
